# revision 3
# baseline (speedup 1.0000x reference)
"""Trainium2 Bass kernel v3 for MAELDRegLoss (LID regularizer via k-NN).

v3 vs v2:
  - PE emits only the 6 feature matmuls per 512-col chunk (y2 no longer rides
    the PE); loop is reordered so one stationary weight tile serves 4 chunks
    (quarter-groups ping-ponging across the 8 PSUM banks) to amortize
    LDWEIGHTS, which v2 evidence suggests serializes with the matmul stream
    (~355ns/MM observed vs ~215ns streaming).
  - ScalarE drains PSUM -> SBUF fp16 (raw 2xy scores, range +-~250).
  - DVE adds the replicated centered -(y2-768) row in-place (fp16 2x mode),
    then max8 per 512-col segment -> 128 candidates; stage 2 as before.
  - Offline sim of this exact pipeline: max rel err 2.4e-3 (gate 2e-2).
"""

import numpy as np

N, D = 8192, 768
NCORES = 8
R = N // NCORES          # 1024 rows per core
RB = 128                 # rows per partition block
NRB = R // RB            # 8 row blocks per core
KT = D // 128            # 6 contraction tiles of 128
NCH = 512                # PSUM chunk columns (one fp32 bank)
NNCH = N // NCH          # 16 chunks per row block
GRP = 4                  # chunks per stationary-reuse group
NGRP = NNCH // GRP       # 4 groups per row block
SEG = 512                # stage-1 max8 segment size == chunk size
NSEG = N // SEG          # 16
CAND = NSEG * 8          # 128 candidates per row
CENTER = 768.0
NEG_BIG = -1.0e30

_cache = {}


def _strip_redundant_mm_incs(nc):
    """Thin the per-matmul semaphore increments in the hardware-loop body.

    Tile emits a sem-inc on every Matmult of the PE counting semaphore, but
    consumers only ever wait on 128 distinct values (one per chunk drain).
    Keep only the incs at referenced positions, remap each wait value to its
    rank among kept positions, and rescale the loop reset/exit/skip
    constants from the old total to the kept count. Gating is unchanged:
    each wait still fires exactly when its original gate matmul completes.
    """
    f = list(nc.m.functions)[0]
    blocks = list(f.blocks)
    # find the counting sem: the one Matmults increment
    from collections import Counter, defaultdict
    sem_mm_incs = Counter()
    for b in blocks:
        for i in b.instructions:
            if i.opcode != "Matmult" or i.sync_info is None:
                continue
            for u in i.sync_info.on_update:
                if u.update_mode == "sem-inc":
                    sem_mm_incs[u.ant_name] += 1
    if not sem_mm_incs:
        return 0
    sem, total = sem_mm_incs.most_common(1)[0]

    # body blocks = those containing the MM incs (expect exactly one)
    body_blocks = []
    for b in blocks:
        n = sum(
            1
            for i in b.instructions
            if i.opcode == "Matmult" and i.sync_info is not None
            and any(u.ant_name == sem and u.update_mode == "sem-inc"
                    for u in i.sync_info.on_update)
        )
        if n:
            body_blocks.append((b, n))
    if len(body_blocks) != 1 or body_blocks[0][1] != total:
        return 0
    body = body_blocks[0][0]

    # referenced positions within the body
    referenced = set()
    for i in body.instructions:
        if i.sync_info is None:
            continue
        for w in i.sync_info.on_wait:
            if w.ant_name == sem:
                if w.wait_mode != "sem-ge-imm" or not (1 <= w.wait_value <= total):
                    return 0
                referenced.add(w.wait_value)
    kept = sorted(referenced | {total})
    rank = {v: k + 1 for k, v in enumerate(kept)}
    keep_set = set(kept)

    # pass 1: thin incs / remap waits inside the body
    pos = 0
    n_stripped = 0
    for i in body.instructions:
        si = i.sync_info
        if i.opcode == "Matmult" and si is not None and any(
            u.ant_name == sem and u.update_mode == "sem-inc"
            for u in si.on_update
        ):
            pos += 1
            if pos not in keep_set:
                si.on_update = [
                    u for u in si.on_update
                    if not (u.ant_name == sem and u.update_mode == "sem-inc")
                ]
                n_stripped += 1
        if si is not None:
            for w in si.on_wait:
                if w.ant_name == sem:
                    w.wait_value = rank[w.wait_value]

    # pass 2: rescale barrier constants in the other blocks
    new_total = len(kept)
    for b in blocks:
        if b.name == body.name:
            continue
        for i in b.instructions:
            si = i.sync_info
            if si is None:
                continue
            for w in si.on_wait:
                if w.ant_name == sem and w.wait_value == total:
                    w.wait_value = new_total
            for u in si.on_update:
                if u.ant_name == sem and u.update_mode in (
                    "sem-add-imm", "sem-sub-imm"
                ) and u.update_value == total:
                    u.update_value = new_total
    return n_stripped



def _build_program(loop_reps=None, ablate=None):
    import concourse.bacc as bacc
    import concourse.tile as tile
    import concourse.mybir as mybir
    from contextlib import ExitStack, nullcontext

    f16 = mybir.dt.float16
    f32 = mybir.dt.float32

    nc = bacc.Bacc("TRN2", target_bir_lowering=False, debug=False)

    lhs_d = nc.declare_dram_parameter("lhs", [D, R], f16, isOutput=False)
    rhs_d = nc.declare_dram_parameter("rhs", [D, N], f16, isOutput=False)
    y2_d = nc.declare_dram_parameter("y2rep", [RB, N], f16, isOutput=False)
    x2_d = nc.declare_dram_parameter("x2", [RB, NRB], f32, isOutput=False)
    out_d = nc.declare_dram_parameter("out", [RB, NRB], f32, isOutput=True)

    with tile.TileContext(nc) as tc, ExitStack() as ctx:
        const_pool = ctx.enter_context(tc.tile_pool(name="const", bufs=1))
        psum_pool = ctx.enter_context(tc.tile_pool(name="psum", bufs=2, space="PSUM"))
        score_pool = ctx.enter_context(tc.tile_pool(name="scores", bufs=2))
        work_pool = ctx.enter_context(tc.tile_pool(name="work", bufs=2))

        rhs_sb = const_pool.tile([RB, KT * N], f16, tag="rhs")
        lhs_sb = const_pool.tile([RB, KT * R], f16, tag="lhs")
        y2_sb = const_pool.tile([RB, N], f16, tag="y2")
        x2_sb = const_pool.tile([RB, NRB], f32, tag="x2")
        outs_sb = const_pool.tile([RB, NRB], f32, tag="outs")

        nc.sync.dma_start(x2_sb[:, :], x2_d[:, :])
        nc.sync.dma_start(y2_sb[:, :], y2_d[:, :])
        for kk in range(KT):
            nc.sync.dma_start(
                lhs_sb[:, kk * R:(kk + 1) * R], lhs_d[kk * RB:(kk + 1) * RB, :]
            )
        for kk in range(KT):
            nc.sync.dma_start(
                rhs_sb[:, kk * N:(kk + 1) * N], rhs_d[kk * RB:(kk + 1) * RB, :]
            )

        if loop_reps is not None:
            loop_cm = tc.For_i(
                0, loop_reps, 1,
                hint_engines=(
                    mybir.EngineType.PE, mybir.EngineType.DVE,
                    mybir.EngineType.Activation, mybir.EngineType.SP,
                    mybir.EngineType.Pool,
                ),
            )
        else:
            loop_cm = nullcontext()
        with loop_cm:
            _emit_body(nc, tc, mybir, work_pool, psum_pool, score_pool, rhs_sb,
                       lhs_sb, y2_sb, x2_sb, outs_sb, ablate)

        nc.sync.dma_start(out_d[:, :], outs_sb[:, :])

    nc.compile()
    if loop_reps is not None:
        _strip_redundant_mm_incs(nc)
    return nc


def _emit_body(nc, tc, mybir, work_pool, psum_pool, score_pool, rhs_sb, lhs_sb,
               y2_sb, x2_sb, outs_sb, ablate=None):
    f16 = mybir.dt.float16
    f32 = mybir.dt.float32
    AF = mybir.ActivationFunctionType
    ALU = mybir.AluOpType
    gemm_only = ablate in ("gemm_only", "gemm_only_noreorder")
    reorder = ablate != "gemm_only_noreorder"

    for rb in range(NRB):
        scores = score_pool.tile([RB, N], f16, tag="scores")
        cand = work_pool.tile([RB, CAND], f16, tag="cand")

        def consume(ch, ps):
            sl = slice(ch * NCH, (ch + 1) * NCH)
            if ablate == "dve_drain":
                # fused drain+add on DVE straight from PSUM
                nc.vector.tensor_tensor(
                    scores[:, sl], ps[:, :], y2_sb[:, sl], op=ALU.add)
            else:
                # drain PSUM -> SBUF fp16 on ScalarE (raw scores)
                nc.scalar.activation(scores[:, sl], ps[:, :], AF.Copy)
                if gemm_only:
                    return
                # DVE: add centered -(y2-768) in place
                nc.vector.tensor_add(scores[:, sl], scores[:, sl], y2_sb[:, sl])
            nc.vector.max(cand[:, ch * 8:(ch + 1) * 8], scores[:, sl])

        if ablate == "drain2048":
            for g in range(NGRP):
                psg = psum_pool.tile([RB, GRP * NCH], f32, tag="psg")
                for kk in range(KT):
                    for c in range(GRP):
                        ch = g * GRP + c
                        nc.tensor.matmul(
                            psg[:, c * NCH:(c + 1) * NCH],
                            lhs_sb[:, kk * R + rb * RB: kk * R + (rb + 1) * RB],
                            rhs_sb[:, kk * N + ch * NCH: kk * N + (ch + 1) * NCH],
                            start=(kk == 0),
                            stop=(kk == KT - 1),
                        )
                gsl = slice(g * GRP * NCH, (g + 1) * GRP * NCH)
                nc.scalar.activation(scores[:, gsl], psg[:, :], AF.Copy)
                nc.vector.tensor_add(scores[:, gsl], scores[:, gsl],
                                     y2_sb[:, gsl])
                for c in range(GRP):
                    ch = g * GRP + c
                    nc.vector.max(cand[:, ch * 8:(ch + 1) * 8],
                                  scores[:, ch * NCH:(ch + 1) * NCH])
        elif reorder:
            for g in range(NGRP):
                pss = [psum_pool.tile([RB, NCH], f32, tag=f"ps{c}",
                                      name=f"ps{c}")
                       for c in range(GRP)]
                for kk in range(KT):
                    for c in range(GRP):
                        ch = g * GRP + c
                        nc.tensor.matmul(
                            pss[c][:, :],
                            lhs_sb[:, kk * R + rb * RB: kk * R + (rb + 1) * RB],
                            rhs_sb[:, kk * N + ch * NCH: kk * N + (ch + 1) * NCH],
                            start=(kk == 0),
                            stop=(kk == KT - 1),
                        )
                for c in range(GRP):
                    if gemm_only and not (g == 0 and c == 0):
                        continue
                    consume(g * GRP + c, pss[c])
        else:
            for ch in range(NNCH):
                ps = psum_pool.tile([RB, NCH], f32, tag="ps", bufs=8)
                for kk in range(KT):
                    nc.tensor.matmul(
                        ps[:, :],
                        lhs_sb[:, kk * R + rb * RB: kk * R + (rb + 1) * RB],
                        rhs_sb[:, kk * N + ch * NCH: kk * N + (ch + 1) * NCH],
                        start=(kk == 0),
                        stop=(kk == KT - 1),
                    )
                if not gemm_only or ch == 0:
                    consume(ch, ps)
        if gemm_only:
            if rb == 0:
                nc.vector.memset(outs_sb[:, :], 0.0)
            continue

        # stage 2: exact top-24 (descending) of the 128 candidates
        t24 = work_pool.tile([RB, 24], f16, tag="t24")
        nc.vector.max(t24[:, 0:8], cand)
        nc.vector.match_replace(cand, t24[:, 0:8], cand, NEG_BIG)
        nc.vector.max(t24[:, 8:16], cand)
        nc.vector.match_replace(cand, t24[:, 8:16], cand, NEG_BIG)
        nc.vector.max(t24[:, 16:24], cand)

        # tail: a = sqrt(max(x2' - s'', 1e-12)), ascending in the free dim
        u = work_pool.tile([RB, 24], f32, tag="u")
        nc.vector.tensor_scalar(
            u[:, :], t24[:, :], -1.0, x2_sb[:, rb:rb + 1],
            op0=ALU.mult, op1=ALU.add,
        )
        nc.vector.tensor_scalar_max(u[:, :], u[:, :], 1e-12)
        a_lut = work_pool.tile([RB, 24], f32, tag="a_lut")
        nc.scalar.activation(a_lut[:, :], u[:, :], AF.Sqrt)
        # one Newton step: a = 0.5 * (a_lut + u / a_lut)
        a_nr = work_pool.tile([RB, 24], f32, tag="a_nr")
        nc.vector.reciprocal(a_nr[:, :], a_lut[:, :])
        nc.vector.tensor_mul(a_nr[:, :], a_nr[:, :], u[:, :])
        nc.vector.tensor_add(a_nr[:, :], a_nr[:, :], a_lut[:, :])
        nc.vector.tensor_scalar_mul(a_nr[:, :], a_nr[:, :], 0.5)

        # m = mean(a[1:20]); denom = a[20] - m; out = -|ln m - ln denom|
        red = work_pool.tile([RB, 4], f32, tag="red")
        nc.vector.tensor_reduce(
            red[:, 0:1], a_nr[:, 1:20], axis=mybir.AxisListType.X, op=ALU.add
        )
        nc.vector.tensor_scalar_mul(red[:, 0:1], red[:, 0:1], 1.0 / 19.0)
        nc.vector.tensor_sub(red[:, 1:2], a_nr[:, 20:21], red[:, 0:1])
        lg = work_pool.tile([RB, 2], f32, tag="lg")
        nc.scalar.activation(lg[:, 0:1], red[:, 0:1], AF.Ln)
        nc.scalar.activation(lg[:, 1:2], red[:, 1:2], AF.Ln)
        nc.vector.tensor_sub(red[:, 2:3], lg[:, 0:1], lg[:, 1:2])
        nc.scalar.activation(red[:, 3:4], red[:, 2:3], AF.Abs)
        nc.vector.tensor_scalar_mul(outs_sb[:, rb:rb + 1], red[:, 3:4], -1.0)


def get_program(loop_reps=None, ablate=None):
    key = ("nc", loop_reps, ablate)
    if key not in _cache:
        _cache[key] = _build_program(loop_reps, ablate)
    return _cache[key]


def make_in_maps(features: np.ndarray):
    F = np.ascontiguousarray(np.asarray(features, dtype=np.float32))
    assert F.shape == (N, D)
    FT = np.ascontiguousarray(F.T)                      # [768, 8192] f32
    rhs16 = (2.0 * FT).astype(np.float16)               # [768, 8192]
    y2 = np.sum(F * F, axis=1, dtype=np.float32)        # [8192]
    y2c = (-(y2 - CENTER)).astype(np.float16)
    y2rep = np.ascontiguousarray(np.broadcast_to(y2c, (RB, N)))
    x2p = (y2 + CENTER).astype(np.float32)              # x2' = x2 + 768
    in_maps = []
    for i in range(NCORES):
        sl = slice(i * R, (i + 1) * R)
        in_maps.append({
            "lhs": np.ascontiguousarray(FT[:, sl]).astype(np.float16),
            "rhs": rhs16,
            "y2rep": y2rep,
            "x2": np.ascontiguousarray(x2p[sl].reshape(NRB, RB).T),
        })
    return in_maps


def kernel(features: np.ndarray, k) -> np.ndarray:
    assert int(k) == 20, f"kernel hardcodes k=20, got {k}"
    from concourse.bass_utils import run_bass_kernel_spmd

    nc = get_program()
    in_maps = make_in_maps(features)
    res = run_bass_kernel_spmd(nc, in_maps, core_ids=list(range(NCORES)))
    out = np.empty((N,), np.float32)
    for i in range(NCORES):
        blk = np.asarray(res.results[i]["out"], np.float32)   # [128, 8]
        out[i * R:(i + 1) * R] = blk.T.reshape(R)
    return out


if __name__ == "__main__":
    import reference

    inputs = reference.setup_inputs()
    expected = np.asarray(reference.reference(**inputs))
    actual = kernel(**{k: np.asarray(v) for k, v in inputs.items()})
    rel = np.abs(actual - expected) / np.maximum(np.abs(expected), 1e-9)
    print("max rel err:", rel.max(), "mean rel err:", rel.mean())
